# revision 7
# baseline (speedup 1.0000x reference)
"""CavityLoss Trainium2 kernel (nn_CavityLoss_43722767073667), v8.

Mathematical reduction of the reference, exact in fp32 (verified against a
bit-faithful numpy emulation incl. adversarial threshold-boundary values):

  pb = (floor(pred*255) >= 128)  <=>  (pred >= c*),  c* = f32(128/255)
  The 5^3 all-ones dilation of the binary gt is an exact integer count
  >= gt (the window contains the center voxel), so
      diff = ((gt - pb*dilate(gt)) > 0) == gt * (1 - pb)     [identity]
  Non-critical voxels contribute exactly 0 to the BCE in fp32, so
      loss = -mean( gt * [pred < c*] * ln(pred) ).

Per-tile chain (v8 "accumulate-on-DVE"):
  STT  (DVE): r    = (p is_ge c*) max p     # p if p < c*, else exactly 1.0
  Ln   (ACT): ln_r = Ln(r)                  # gated by pred only!
  TTR  (DVE): acc[:,k] = sum(gt * ln_r)     # gt==0 rows contribute exact 0;
                                            # gt==1, p>=c*: Ln_table(1.0)<=4e-7
  (residual: eps * 6.0M / 1.79M <= ~1.3e-6 relative, far under the 2e-2 gate)

v8 changes (from HW trace analysis of v6 @35.6us / v7 @36.7us):
  - SDMA engine 15 is persistently ~13% slower than engines 0-14 (measured
    21.4 vs 24.7 B/ns in both v6 and v7 traces) and finishes ~3-4us after
    the rest, gating every transfer's completion semaphore. Engine 15 serves
    partitions {92-95, 124-127} (fixed HW swizzle). We therefore give those
    8 partitions only 6042 columns while the other 120 partitions get 6970:
    the extra 928-col region is DMAd as two rectangles ([0:92] and [96:124])
    that engine 15 never touches. Its dead SBUF rows are pre-memset
    (pred=1.0 -> r=1.0 -> Ln~0; gt=0.0 -> product exact 0). Engine loads:
    eng15 8*6042*8B = 387KB @21.4 = 18.1us; eng0-14 8*6970*8B = 446KB
    @24.7 = 18.1us -- balanced, removing the ~2.5us straggler tail.
  - Row-sum accumulation moved from ACT (accum_out + 280ns READ_ACCUMULATOR
    per tile, serializing the tail) to DVE tensor_tensor_reduce. ACT's Ln
    work is pred-gated so it all lands mid-stream; the post-stream tail is
    only the last TTRs plus one [128,NT] DMA that ACT issues on its own
    HWDGE ring (no cross-engine finalize hops, no PE/PSUM).
  - Pred tiles lead their gt partner by ~2 slots in the transfer order so
    STT+Ln complete before the ttr's gt operand arrives; descending tail
    tile sizes keep the last ttr small.

Distribution: 192^3 volume flattened and split into 8 equal slabs (depth
sharding: 24 z-planes per core). Pointwise + reduction only - the dilation
cancels, so no halo exchange and no collectives.
"""

import numpy as np

import concourse.bacc as bacc
import concourse.mybir as mybir
from concourse.bass_utils import run_bass_kernel_spmd

D = 192
N_CORES = 8
P = 128
TOTAL = D * D * D              # 7_077_888
PER_CORE = TOTAL // N_CORES    # 884_736

# --- engine-15-balanced layout -------------------------------------------
W_S = 6042                     # columns on slow partitions {92-95,124-127}
W_X = 928                      # extra columns on the 120 fast partitions
W_F = W_S + W_X                # 6970
N_FAST = 120
assert P * W_S + N_FAST * W_X == PER_CORE
SLOW_ROWS = ((92, 96), (124, 128))

U_SIZES = [1792, 1792, 1280, 768, 410]   # uniform-region compute tiles
assert sum(U_SIZES) == W_S
# compute-tile order: U0, E(the 928-col fast-only region), U1, U2, U3, U4
C_STAR = float(np.float32(128.0) / np.float32(255.0))

_CACHE = {}


def _build():
    nc = bacc.Bacc("TRN2", name="cavity_loss")
    f32 = mybir.dt.float32
    pred_u = nc.dram_tensor("pred_u", [P, W_S], f32, kind="ExternalInput")
    gt_u = nc.dram_tensor("gt_u", [P, W_S], f32, kind="ExternalInput")
    pred_x = nc.dram_tensor("pred_x", [N_FAST, W_X], f32, kind="ExternalInput")
    gt_x = nc.dram_tensor("gt_x", [N_FAST, W_X], f32, kind="ExternalInput")

    ge = mybir.AluOpType.is_ge
    mult = mybir.AluOpType.mult
    add = mybir.AluOpType.add
    mx = mybir.AluOpType.max
    Ln = mybir.ActivationFunctionType.Ln

    pred_sb = nc.alloc_sbuf_tensor("pred_sb", [P, W_F], f32).ap()
    gt_sb = nc.alloc_sbuf_tensor("gt_sb", [P, W_F], f32).ap()
    r_sb = nc.alloc_sbuf_tensor("r_sb", [P, W_F], f32).ap()
    ln_sb = nc.alloc_sbuf_tensor("ln_sb", [P, W_F], f32).ap()

    # compute tiles: (key, col slice); E is the fast-only extra region
    offs = np.concatenate([[0], np.cumsum(U_SIZES)]).tolist()
    usl = [slice(offs[t], offs[t + 1]) for t in range(len(U_SIZES))]
    xsl = slice(W_S, W_F)
    tiles = [("U0", usl[0]), ("E", xsl), ("U1", usl[1]), ("U2", usl[2]),
             ("U3", usl[3]), ("U4", usl[4])]
    NT = len(tiles)
    acc = nc.alloc_sbuf_tensor("acc_sb", [P, NT], f32).ap()
    out = nc.dram_tensor("out", [P, NT], f32, kind="ExternalOutput")

    s_p = {k: nc.alloc_semaphore(f"s_p_{k}") for k, _ in tiles}
    s_g = {k: nc.alloc_semaphore(f"s_g_{k}") for k, _ in tiles}
    s_px1 = nc.alloc_semaphore("s_px1")   # second rect of the E pred DMA
    s_gx1 = nc.alloc_semaphore("s_gx1")
    s_init = nc.alloc_semaphore("s_init")
    s_r1 = nc.alloc_semaphore("s_r1")
    s_ln = nc.alloc_semaphore("s_ln")
    s_fin = nc.alloc_semaphore("s_fin")
    s_out = nc.alloc_semaphore("s_out")

    # --- gpsimd: fill engine-15's dead rows of the extra region -----------
    # pred=1.0 -> r=1.0 -> Ln(1.0) ~ 0; gt=0.0 -> ttr product exact 0.
    # Compute-engine partition bases must be quadrant-aligned, so memset the
    # whole [64:128] range; the E-region DMAs (issued after s_init fires)
    # overwrite the live rows with real data.
    nc.gpsimd.memset(pred_sb[64:128, xsl], 1.0)
    nc.gpsimd.memset(gt_sb[64:128, xsl], 0.0).then_inc(s_init, 1)

    # --- sync HWDGE ring: pred leads its gt partner by ~2 slots -----------
    def dma(dst, src, sem):
        nc.sync.dma_start(dst, src).then_inc(sem, 16)

    def dma_u(t, which):
        sl = usl[t]
        if which == "p":
            dma(pred_sb[:, sl], pred_u[:, sl], s_p[f"U{t}"])
        else:
            dma(gt_sb[:, sl], gt_u[:, sl], s_g[f"U{t}"])

    dma_u(0, "p")                                          # pU0
    nc.sync.wait_ge(s_init, 1)   # E-region DMAs land after the row fills
    dma(pred_sb[0:92, xsl], pred_x[0:92, :], s_p["E"])     # pE0
    dma(pred_sb[96:124, xsl], pred_x[92:120, :], s_px1)    # pE1
    dma_u(1, "p")                                          # pU1
    dma_u(0, "g")                                          # gU0
    dma_u(2, "p")                                          # pU2
    dma(gt_sb[0:92, xsl], gt_x[0:92, :], s_g["E"])         # gE0
    dma(gt_sb[96:124, xsl], gt_x[92:120, :], s_gx1)        # gE1
    dma_u(1, "g")                                          # gU1
    dma_u(3, "p")                                          # pU3
    dma_u(2, "g")                                          # gU2
    dma_u(4, "p")                                          # pU4
    dma_u(3, "g")                                          # gU3
    dma_u(4, "g")                                          # gU4

    # --- scalar (ACT): table-load hoist + pred-gated Ln chain -------------
    dummy = nc.alloc_sbuf_tensor("dummy_sb", [P, 1], f32).ap()
    nc.scalar.activation(dummy[:], nc.const_aps.tensor(1.0, (P, 1)), Ln)
    for k, (key, sl) in enumerate(tiles):
        nc.scalar.wait_ge(s_r1, k + 1)
        nc.scalar.activation(ln_sb[:, sl], r_sb[:, sl], Ln).then_inc(s_ln, 1)
    # finalize on ACT's own HWDGE ring right after the last ttr lands
    nc.scalar.wait_ge(s_fin, 1)
    nc.scalar.dma_start(out[:], acc[:]).then_inc(s_out, 16)
    nc.scalar.wait_ge(s_out, 16)

    # --- vector (DVE): STT (pred-keyed) + TTR (gt-keyed, accumulates) -----
    def stt(k):
        key, sl = tiles[k]
        if key == "E":
            nc.vector.wait_ge(s_init, 1)
            nc.vector.wait_ge(s_px1, 16)
        nc.vector.wait_ge(s_p[key], 16)
        nc.vector.scalar_tensor_tensor(
            r_sb[:, sl], pred_sb[:, sl], C_STAR, pred_sb[:, sl], ge, mx
        ).then_inc(s_r1, 1)

    def ttr(k):
        # (tensor_tensor_reduce faults this runtime; STT with accum_out is
        # the HW-validated equivalent: out = (gt bypass 0) mult ln, acc=sum)
        key, sl = tiles[k]
        if key == "E":
            nc.vector.wait_ge(s_gx1, 16)
        nc.vector.wait_ge(s_g[key], 16)
        nc.vector.wait_ge(s_ln, k + 1)
        t = nc.vector.scalar_tensor_tensor(
            r_sb[:, sl], gt_sb[:, sl], 0.0, ln_sb[:, sl],
            mybir.AluOpType.bypass, mult, accum_out=acc[:, k : k + 1],
        )
        return t

    # interleaved to match arrival order (see transfer order above)
    stt(0)            # U0
    stt(1)            # E
    stt(2)            # U1
    ttr(0)            # U0
    stt(3)            # U2
    ttr(1)            # E
    ttr(2)            # U1
    stt(4)            # U3
    ttr(3)            # U2
    stt(5)            # U4
    ttr(4)            # U3
    ttr(5).then_inc(s_fin, 1)   # U4

    nc.compile()
    return nc


def _get_nc():
    if "nc" not in _CACHE:
        _CACHE["nc"] = _build()
    return _CACHE["nc"]


def _shard(x):
    """Split into 8 slabs; pack each as (uniform [128,W_S], extra [120,W_X])."""
    flat = np.ascontiguousarray(np.asarray(x, dtype=np.float32)).reshape(-1)
    assert flat.size == TOTAL, f"expected {TOTAL} elements, got {flat.size}"
    outs = []
    for c in range(N_CORES):
        slab = flat[c * PER_CORE : (c + 1) * PER_CORE]
        u = slab[: P * W_S].reshape(P, W_S)
        xtra = slab[P * W_S :].reshape(N_FAST, W_X)
        outs.append((u, xtra))
    return outs


def run_spmd(pred, gt, **kw):
    """Shard, run on 8 cores; returns BassKernelResults (kw e.g. trace=True)."""
    preds = _shard(pred)
    gts = _shard(gt)
    in_maps = [
        {
            "pred_u": preds[c][0],
            "pred_x": preds[c][1],
            "gt_u": gts[c][0],
            "gt_x": gts[c][1],
        }
        for c in range(N_CORES)
    ]
    return run_bass_kernel_spmd(
        _get_nc(), in_maps, core_ids=list(range(N_CORES)), **kw
    )


def kernel(pred, gt):
    res = run_spmd(pred, gt)
    total = 0.0
    for r in res.results:
        total += float(r["out"].astype(np.float64).sum())
    return np.asarray(np.float32(-total / TOTAL))


# revision 8
# speedup vs baseline: 1.2199x; 1.2199x over previous
"""CavityLoss Trainium2 kernel (nn_CavityLoss_43722767073667), v9.

Mathematical reduction of the reference, exact in fp32 (verified against a
bit-faithful numpy emulation incl. adversarial threshold-boundary values):

  pb = (floor(pred*255) >= 128)  <=>  (pred >= c*),  c* = f32(128/255)
  The 5^3 all-ones dilation of the binary gt is an exact integer count
  >= gt (the window contains the center voxel), so
      diff = ((gt - pb*dilate(gt)) > 0) == gt * (1 - pb)     [identity]
  Non-critical voxels contribute exactly 0 to the BCE in fp32, so
      loss = -mean( gt * [pred < c*] * ln(pred) ).

Per-tile chain (v9 "accumulate-on-DVE"):
  STT#1 (DVE): r    = (p is_ge c*) max p    # p if p < c*, else exactly 1.0
  Ln    (ACT): ln_r = Ln(r)                 # gated by pred only
  STT#2 (DVE): acc[:,k] = sum((gt bypass 0) mult ln_r)   # accum_out row-sum
  gt==0 voxels contribute exact 0; gt==1, p>=c* contribute Ln_table(1.0)
  (<=4e-7); residual <= ~1.3e-6 relative, far under the 2e-2 gate.

v9 design notes (from HW traces of v6 @35.6us, v7 @36.7us, v8 @41.2us):
  - The measured window is [first const-memset .. last walrus postamble
    instruction]; the ~7us all-semaphore-reset postamble and ~1us preamble
    are fixed, so only the body (lead-in + stream + tail) is optimizable.
  - The input stream is HBM/DMA-engine bound; SDMA engine 15 is persistently
    ~13% slower (21.4 vs 24.7 B/ns) and its FIFO backlog sets every
    transfer's completion-sem time (~1.5ns/col cumulative). Partial-partition
    DMAs (to starve engine 15's partitions) backfire on BOTH DGE paths: the
    descriptor->engine assignment chunks onto 4 engines and the whole stream
    slows ~25% (v8). So the layout stays uniform [128, 6912].
  - ACT work (Ln) is pred-gated: with pred tiles leading their gt partner by
    2 transfer slots, every Ln lands mid-stream; the post-stream tail is
    just the last accumulating STTs. No ACT READ_ACCUMULATOR chain (v6),
    no PE matmul / PSUM copy: ACT itself DMAs acc[128,NT] on its own HWDGE
    ring right after the last accum lands; host reduces in f64.
  - Tile sizes [2560,1728,1152,832,640] from a discrete-event model of the
    tail (eng15 clock, DVE/ACT serialization, measured op costs): descending
    ramp so each accum-STT finishes before the next gt sem fires.

Distribution: 192^3 volume flattened and split into 8 equal slabs (depth
sharding: 24 z-planes per core), each viewed as [128 partitions, 6912].
Pointwise + reduction only - the dilation cancels, so no halo exchange and
no collectives.
"""

import numpy as np

import concourse.bacc as bacc
import concourse.mybir as mybir
from concourse.bass_utils import run_bass_kernel_spmd

D = 192
N_CORES = 8
P = 128
TOTAL = D * D * D              # 7_077_888
PER_CORE = TOTAL // N_CORES    # 884_736
FREE = PER_CORE // P           # 6_912
SIZES = [2560, 1728, 1152, 832, 640]
assert sum(SIZES) == FREE
NT = len(SIZES)

C_STAR = float(np.float32(128.0) / np.float32(255.0))

_CACHE = {}


def _build():
    nc = bacc.Bacc("TRN2", name="cavity_loss")
    f32 = mybir.dt.float32
    pred = nc.dram_tensor("pred", [P, FREE], f32, kind="ExternalInput")
    gt = nc.dram_tensor("gt", [P, FREE], f32, kind="ExternalInput")
    out = nc.dram_tensor("out", [P, NT], f32, kind="ExternalOutput")

    ge = mybir.AluOpType.is_ge
    byp = mybir.AluOpType.bypass
    mult = mybir.AluOpType.mult
    mx = mybir.AluOpType.max
    Ln = mybir.ActivationFunctionType.Ln

    pred_sb = nc.alloc_sbuf_tensor("pred_sb", [P, FREE], f32).ap()
    gt_sb = nc.alloc_sbuf_tensor("gt_sb", [P, FREE], f32).ap()
    r_sb = nc.alloc_sbuf_tensor("r_sb", [P, FREE], f32).ap()
    ln_sb = nc.alloc_sbuf_tensor("ln_sb", [P, FREE], f32).ap()
    acc = nc.alloc_sbuf_tensor("acc_sb", [P, NT], f32).ap()

    s_pred = [nc.alloc_semaphore(f"s_pred{t}") for t in range(NT)]
    s_gt = [nc.alloc_semaphore(f"s_gt{t}") for t in range(NT)]
    s_r1 = nc.alloc_semaphore("s_r1")
    s_ln = nc.alloc_semaphore("s_ln")
    s_fin = nc.alloc_semaphore("s_fin")
    s_out = nc.alloc_semaphore("s_out")

    offs = np.concatenate([[0], np.cumsum(SIZES)]).tolist()
    sls = [slice(offs[t], offs[t + 1]) for t in range(NT)]

    # single sync HWDGE ring; pred tiles lead their gt partner by 2 slots so
    # STT#1+Ln always complete before the gt-gated accumulating STT#2
    def dma_p(t):
        nc.sync.dma_start(pred_sb[:, sls[t]], pred[:, sls[t]]).then_inc(
            s_pred[t], 16
        )

    def dma_g(t):
        nc.sync.dma_start(gt_sb[:, sls[t]], gt[:, sls[t]]).then_inc(s_gt[t], 16)

    dma_p(0)
    dma_p(1)
    for t in range(2, NT):
        dma_g(t - 2)
        dma_p(t)
    dma_g(NT - 2)
    dma_g(NT - 1)

    # scalar (ACT): dummy Ln hoists the ACT_TABLE_LOAD into the DMA window,
    # then the pred-gated Ln chain; finalize DMA on ACT's own HWDGE ring
    dummy = nc.alloc_sbuf_tensor("dummy_sb", [P, 1], f32).ap()
    nc.scalar.activation(dummy[:], nc.const_aps.tensor(1.0, (P, 1)), Ln)
    for t in range(NT):
        nc.scalar.wait_ge(s_r1, t + 1)
        nc.scalar.activation(ln_sb[:, sls[t]], r_sb[:, sls[t]], Ln).then_inc(
            s_ln, 1
        )
    nc.scalar.wait_ge(s_fin, 1)
    nc.scalar.dma_start(out[:], acc[:]).then_inc(s_out, 16)
    nc.scalar.wait_ge(s_out, 16)

    # vector (DVE): STT#1 (pred-keyed) and accumulating STT#2 (gt-keyed)
    def stt1(t):
        sl = sls[t]
        nc.vector.wait_ge(s_pred[t], 16)
        nc.vector.scalar_tensor_tensor(
            r_sb[:, sl], pred_sb[:, sl], C_STAR, pred_sb[:, sl], ge, mx
        ).then_inc(s_r1, 1)

    def stt2(t):
        sl = sls[t]
        nc.vector.wait_ge(s_gt[t], 16)
        nc.vector.wait_ge(s_ln, t + 1)
        return nc.vector.scalar_tensor_tensor(
            r_sb[:, sl], gt_sb[:, sl], 0.0, ln_sb[:, sl], byp, mult,
            accum_out=acc[:, t : t + 1],
        )

    stt1(0)
    stt1(1)
    stt2(0)
    for t in range(2, NT):
        stt1(t)
        stt2(t - 1)
    stt2(NT - 1).then_inc(s_fin, 1)

    nc.compile()
    return nc


def _get_nc():
    if "nc" not in _CACHE:
        _CACHE["nc"] = _build()
    return _CACHE["nc"]


def _shard(x):
    flat = np.ascontiguousarray(np.asarray(x, dtype=np.float32)).reshape(-1)
    assert flat.size == TOTAL, f"expected {TOTAL} elements, got {flat.size}"
    return [
        flat[c * PER_CORE : (c + 1) * PER_CORE].reshape(P, FREE)
        for c in range(N_CORES)
    ]


def run_spmd(pred, gt, **kw):
    """Shard, run on 8 cores; returns BassKernelResults (kw e.g. trace=True)."""
    preds = _shard(pred)
    gts = _shard(gt)
    in_maps = [{"pred": preds[c], "gt": gts[c]} for c in range(N_CORES)]
    return run_bass_kernel_spmd(
        _get_nc(), in_maps, core_ids=list(range(N_CORES)), **kw
    )


def kernel(pred, gt):
    res = run_spmd(pred, gt)
    total = 0.0
    for r in res.results:
        total += float(r["out"].astype(np.float64).sum())
    return np.asarray(np.float32(-total / TOTAL))


# revision 9
# speedup vs baseline: 1.2466x; 1.0219x over previous
"""CavityLoss Trainium2 kernel (nn_CavityLoss_43722767073667), v9.

Mathematical reduction of the reference, exact in fp32 (verified against a
bit-faithful numpy emulation incl. adversarial threshold-boundary values):

  pb = (floor(pred*255) >= 128)  <=>  (pred >= c*),  c* = f32(128/255)
  The 5^3 all-ones dilation of the binary gt is an exact integer count
  >= gt (the window contains the center voxel), so
      diff = ((gt - pb*dilate(gt)) > 0) == gt * (1 - pb)     [identity]
  Non-critical voxels contribute exactly 0 to the BCE in fp32, so
      loss = -mean( gt * [pred < c*] * ln(pred) ).

Per-tile chain (v9 "accumulate-on-DVE"):
  STT#1 (DVE): r    = (p is_ge c*) max p    # p if p < c*, else exactly 1.0
  Ln    (ACT): ln_r = Ln(r)                 # gated by pred only
  STT#2 (DVE): acc[:,k] = sum((gt bypass 0) mult ln_r)   # accum_out row-sum
  gt==0 voxels contribute exact 0; gt==1, p>=c* contribute Ln_table(1.0)
  (<=4e-7); residual <= ~1.3e-6 relative, far under the 2e-2 gate.

v9 design notes (from HW traces of v6 @35.6us, v7 @36.7us, v8 @41.2us):
  - The measured window is [first const-memset .. last walrus postamble
    instruction]; the ~7us all-semaphore-reset postamble and ~1us preamble
    are fixed, so only the body (lead-in + stream + tail) is optimizable.
  - The input stream is HBM/DMA-engine bound; SDMA engine 15 is persistently
    ~13% slower (21.4 vs 24.7 B/ns) and its FIFO backlog sets every
    transfer's completion-sem time (~1.5ns/col cumulative). Partial-partition
    DMAs (to starve engine 15's partitions) backfire on BOTH DGE paths: the
    descriptor->engine assignment chunks onto 4 engines and the whole stream
    slows ~25% (v8). So the layout stays uniform [128, 6912].
  - ACT work (Ln) is pred-gated: with pred tiles leading their gt partner by
    2 transfer slots, every Ln lands mid-stream; the post-stream tail is
    just the last accumulating STTs. No ACT READ_ACCUMULATOR chain (v6),
    no PE matmul / PSUM copy: ACT itself DMAs acc[128,NT] on its own HWDGE
    ring right after the last accum lands; host reduces in f64.
  - Tile sizes [2560,1728,1152,832,640] from a discrete-event model of the
    tail (eng15 clock, DVE/ACT serialization, measured op costs): descending
    ramp so each accum-STT finishes before the next gt sem fires.

Distribution: 192^3 volume flattened and split into 8 equal slabs (depth
sharding: 24 z-planes per core), each viewed as [128 partitions, 6912].
Pointwise + reduction only - the dilation cancels, so no halo exchange and
no collectives.
"""

import numpy as np

import concourse.bacc as bacc
import concourse.mybir as mybir
from concourse.bass_utils import run_bass_kernel_spmd

D = 192
N_CORES = 8
P = 128
TOTAL = D * D * D              # 7_077_888
PER_CORE = TOTAL // N_CORES    # 884_736
FREE = PER_CORE // P           # 6_912
SIZES = [2560, 1728, 1152, 832, 640]
assert sum(SIZES) == FREE
NT = len(SIZES)

C_STAR = float(np.float32(128.0) / np.float32(255.0))

_CACHE = {}


def _build():
    nc = bacc.Bacc("TRN2", name="cavity_loss")
    f32 = mybir.dt.float32
    pred = nc.dram_tensor("pred", [P, FREE], f32, kind="ExternalInput")
    gt = nc.dram_tensor("gt", [P, FREE], f32, kind="ExternalInput")
    out = nc.dram_tensor("out", [P, NT], f32, kind="ExternalOutput")

    ge = mybir.AluOpType.is_ge
    byp = mybir.AluOpType.bypass
    mult = mybir.AluOpType.mult
    mx = mybir.AluOpType.max
    Ln = mybir.ActivationFunctionType.Ln

    pred_sb = nc.alloc_sbuf_tensor("pred_sb", [P, FREE], f32).ap()
    gt_sb = nc.alloc_sbuf_tensor("gt_sb", [P, FREE], f32).ap()
    r_sb = nc.alloc_sbuf_tensor("r_sb", [P, FREE], f32).ap()
    ln_sb = nc.alloc_sbuf_tensor("ln_sb", [P, FREE], f32).ap()
    acc = nc.alloc_sbuf_tensor("acc_sb", [P, NT], f32).ap()

    s_pred = [nc.alloc_semaphore(f"s_pred{t}") for t in range(NT)]
    s_gt = [nc.alloc_semaphore(f"s_gt{t}") for t in range(NT)]
    s_r1 = nc.alloc_semaphore("s_r1")
    s_ln = nc.alloc_semaphore("s_ln")
    s_fin = nc.alloc_semaphore("s_fin")
    s_out = nc.alloc_semaphore("s_out")

    offs = np.concatenate([[0], np.cumsum(SIZES)]).tolist()
    sls = [slice(offs[t], offs[t + 1]) for t in range(NT)]

    # single sync HWDGE ring; pred tiles lead their gt partner by 2 slots so
    # STT#1+Ln always complete before the gt-gated accumulating STT#2
    def dma_p(t):
        nc.sync.dma_start(pred_sb[:, sls[t]], pred[:, sls[t]]).then_inc(
            s_pred[t], 16
        )

    def dma_g(t):
        nc.sync.dma_start(gt_sb[:, sls[t]], gt[:, sls[t]]).then_inc(s_gt[t], 16)

    dma_p(0)
    dma_p(1)
    for t in range(2, NT):
        dma_g(t - 2)
        dma_p(t)
    dma_g(NT - 2)
    dma_g(NT - 1)

    # scalar (ACT): dummy Ln hoists the ACT_TABLE_LOAD into the DMA window,
    # then the pred-gated Ln chain; finalize DMA on ACT's own HWDGE ring
    dummy = nc.alloc_sbuf_tensor("dummy_sb", [P, 1], f32).ap()
    nc.scalar.activation(dummy[:], nc.const_aps.tensor(1.0, (P, 1)), Ln)
    for t in range(NT):
        nc.scalar.wait_ge(s_r1, t + 1)
        nc.scalar.activation(ln_sb[:, sls[t]], r_sb[:, sls[t]], Ln).then_inc(
            s_ln, 1
        )
    nc.scalar.wait_ge(s_fin, 1)
    # no wait on s_out: nothing consumes it, and the walrus postamble (the
    # ~6us all-semaphore-reset chain + final drain/barrier) outlasts the
    # 2.5KB out-DMA landing by several us, so the write is safely retired
    # before NEFF completion; dropping the wait starts the postamble ~1.5us
    # earlier, inside the measured window
    nc.scalar.dma_start(out[:], acc[:]).then_inc(s_out, 16)

    # vector (DVE): STT#1 (pred-keyed) and accumulating STT#2 (gt-keyed)
    def stt1(t):
        sl = sls[t]
        nc.vector.wait_ge(s_pred[t], 16)
        nc.vector.scalar_tensor_tensor(
            r_sb[:, sl], pred_sb[:, sl], C_STAR, pred_sb[:, sl], ge, mx
        ).then_inc(s_r1, 1)

    def stt2(t):
        sl = sls[t]
        nc.vector.wait_ge(s_gt[t], 16)
        nc.vector.wait_ge(s_ln, t + 1)
        return nc.vector.scalar_tensor_tensor(
            r_sb[:, sl], gt_sb[:, sl], 0.0, ln_sb[:, sl], byp, mult,
            accum_out=acc[:, t : t + 1],
        )

    stt1(0)
    stt1(1)
    stt2(0)
    for t in range(2, NT):
        stt1(t)
        stt2(t - 1)
    stt2(NT - 1).then_inc(s_fin, 1)

    nc.compile()
    return nc


def _get_nc():
    if "nc" not in _CACHE:
        _CACHE["nc"] = _build()
    return _CACHE["nc"]


def _shard(x):
    flat = np.ascontiguousarray(np.asarray(x, dtype=np.float32)).reshape(-1)
    assert flat.size == TOTAL, f"expected {TOTAL} elements, got {flat.size}"
    return [
        flat[c * PER_CORE : (c + 1) * PER_CORE].reshape(P, FREE)
        for c in range(N_CORES)
    ]


def run_spmd(pred, gt, **kw):
    """Shard, run on 8 cores; returns BassKernelResults (kw e.g. trace=True)."""
    preds = _shard(pred)
    gts = _shard(gt)
    in_maps = [{"pred": preds[c], "gt": gts[c]} for c in range(N_CORES)]
    return run_bass_kernel_spmd(
        _get_nc(), in_maps, core_ids=list(range(N_CORES)), **kw
    )


def kernel(pred, gt):
    res = run_spmd(pred, gt)
    total = 0.0
    for r in res.results:
        total += float(r["out"].astype(np.float64).sum())
    return np.asarray(np.float32(-total / TOTAL))
